# revision 1
# baseline (speedup 1.0000x reference)
"""Trainium2 Bass kernel for nn_BasicBlock_1w4a_LUT (binarized 3x3 conv + LUT bucketize).

Data-parallel over batch: 8 NeuronCores x 4 images each; no cross-core
communication. Full inputs in, full output out; shard/unshard on the host.

Host prep:
  - Binarize the weights exactly as the reference does; the result is
    sign(bw)*sw with sw>0 per out-channel. sw is folded into the LUT
    thresholds so the device weights are exactly +/-1 (exact in fp16).
  - Split x into hi/lo fp16 (x == hi + lo to ~2^-21 relative; the PE
    multiplies fp16 exactly, verified on HW) and zero-pad each image into a
    "flat" 114x114 slab: partitions 0-63 hi, 64-127 lo. The 3x3/pad-1 conv
    then becomes 9 shifted dot products over the flat vector; the 2 junk
    columns per row at the wrap seams are computed anyway and stripped on
    the host.
  - Per-channel affine z = y*s + b chosen so bucketize thresholds map to
    tau3 -> 0 and tau5 -> 1 (frees DVE scalar slots; the DVE op set below
    only has 3 per-partition scalar slots per instruction).

Device, per image:
  - 9 accumulated K=128 fp16 matmuls per 456-pixel chunk (K packs hi+lo of
    one tap; weights duplicated across the two halves). Two chunks run
    concurrently on the PE via column tiling (tile_position), which also
    fills all 128 PSUM partitions for the vector ops. Runs at the PE
    streaming bound: 1 column/cycle/pair at 2.4 GHz.
  - ScalarE applies the per-channel affine out of PSUM; two custom DVE ops
    (registered at import time into concourse's custom-DVE table) compute
    out = sum_k [z > tau_k] over all 7 thresholds in 2 passes, writing u8.
  - A few junk matmuls at kernel start warm the PE HAM clock gate
    (1.2 -> 2.4 GHz) while the first input DMA pieces land; input/output
    DMAs are split so compute starts/finishes without waiting for whole
    images.
"""

import numpy as np

# ---- problem constants (hardcoded per contract) ----
B, Cin, Cout, H, W = 32, 64, 64, 112, 112
NCORES = 8
BPC = B // NCORES          # images per core
HP = H + 2                 # 114 padded rows
WPAD = W + 2               # 114 padded cols
FLAT = HP * WPAD           # 12996 padded image size
HALF = H * W // 2          # 6272 output pixels per column-tile half
NCH = 4 * W                # 448 pixels per chunk = 4 image rows (PSUM <= 2KB)
NPAIR = H // 8             # 14 adjacent chunk pairs per image
SLABF = HP * WPAD          # 12996: the whole padded image is the slab
NTAPS = 9
NSPLIT = 4                 # input slab DMA pieces per image (after the first)
NWARM = 13                 # PE warm-up matmuls

_built = []
last_results = None


def _register_dve_ops():
    from concourse.dve_spec import (
        Spec, Src0, Src1, C0, C1, C3, One, Zero, lower,
        _spill_c3_to_src1, _has_src1,
    )
    import concourse.dve_ops as dve_ops
    from concourse.dve_ops import DveOp
    from concourse.dve_uop import DveOpSpec

    def register_op(name, spec):
        if name in dve_ops._SUB_OPCODE_FOR_NAME:
            for op in dve_ops.OPS:
                if op.name == name:
                    return op
            raise RuntimeError(name)
        row = max(dve_ops._SUB_OPCODE_FOR_NAME.values()) + 1
        assert row < 0x20, "custom-DVE opcode rows exhausted"
        shas = {}
        for ver in ("v3", "v4"):
            s = DveOpSpec(name=name, opcode=row, uops=lower(spec, ver=ver),
                          rd1_en=_has_src1(spec))
            shas[ver] = s.sha(ver)
        op = DveOp(name, spec, subdim=False, uops_sha=shas)
        dve_ops.OPS.append(op)
        dve_ops.CUSTOM_DVE_SPECS[name] = spec
        dve_ops._SUB_OPCODE_FOR_NAME[name] = row
        return op

    # u = (z>tau0) + (z>tau1) + (z>tau2);  tau2 rides C3 (spilled to in1 [P,1])
    bucket3 = register_op(
        "BUCKET3_ANT",
        Spec(
            body=_spill_c3_to_src1(((Src0 > C0) + (Src0 > C1)) + (Src0 > C3)),
            reference=lambda in0, in1, s0, s1, imm2: (
                (in0 > s0).astype(np.float32) + (in0 > s1)
                + (in0 > np.asarray(in1, np.float32).reshape(-1, 1))
            ),
        ),
    )
    # out = (z>0) + (z>1) + (z>tau4) + (z>tau6) + u
    bucket4acc = register_op(
        "BUCKET4ACC_ANT",
        Spec(
            body=(((Src0 > Zero) + (Src0 > One))
                  + ((Src0 > C0) + (Src0 > C1))) + Src1,
            reference=lambda in0, in1, s0, s1, imm2: (
                (in0 > 0).astype(np.float32) + (in0 > 1)
                + (in0 > s0) + (in0 > s1) + in1
            ),
        ),
    )
    return bucket3, bucket4acc


def _build():
    """Trace + compile the per-core Bass kernel (once per process)."""
    if _built:
        return _built[0]

    import concourse.bacc as bacc
    import concourse.mybir as mybir
    import concourse.tile as tile

    bucket3, bucket4acc = _register_dve_ops()

    f32, f16, u8 = mybir.dt.float32, mybir.dt.float16, mybir.dt.uint8
    nc = bacc.Bacc("TRN2", target_bir_lowering=False, debug=False,
                   num_devices=NCORES)

    xin_t = nc.dram_tensor("xin", [BPC, 128, SLABF], f16, kind="ExternalInput")
    wts_t = nc.dram_tensor("wts", [128, NTAPS, Cout], f16, kind="ExternalInput")
    nrm_t = nc.dram_tensor("nrm", [128, 7], f32, kind="ExternalInput")
    out_t = nc.dram_tensor("out", [BPC, 128, HALF], u8, kind="ExternalOutput")

    with tile.TileContext(nc) as tc:
        with (
            tc.tile_pool(name="const", bufs=1) as cpool,
            tc.tile_pool(name="slab", bufs=2) as spool,
            tc.tile_pool(name="psum", bufs=6, space="PSUM") as ppool,
            tc.tile_pool(name="wpsum", bufs=1, space="PSUM") as wpool,
            tc.tile_pool(name="z", bufs=4) as zpool,
            tc.tile_pool(name="u", bufs=4) as upool,
            tc.tile_pool(name="o", bufs=2) as opool,
        ):
            # first slab piece of image 0 goes out before anything else — it
            # gates the first matmuls
            slab0 = spool.tile([128, SLABF], f16, tag="slab")
            nc.sync.dma_start(out=slab0[:, 0:1400], in_=xin_t.ap()[0, :, 0:1400])
            wts = cpool.tile([128, NTAPS, Cout], f16)
            nc.scalar.dma_start(out=wts[:], in_=wts_t.ap())
            nrm = cpool.tile([128, 7], f32)
            nc.scalar.dma_start(out=nrm[:], in_=nrm_t.ap())

            # PE warm-up: junk matmuls on a zeroed tile while the first input
            # DMAs land, so the HAM clock gate opens (1.2 -> 2.4 GHz) before
            # the real matmuls start.
            wu = cpool.tile([128, NCH], f16)
            nc.gpsimd.memset(wu[:], 0.0)
            wps = wpool.tile([64, NCH], f32)
            for _ in range(NWARM):
                nc.tensor.matmul(wps[:], wu[:, 0:Cout], wu[:],
                                 tile_position=(0, 0), start=True, stop=True)

            scale, bias = nrm[:, 0:1], nrm[:, 1:2]
            tau0, tau1, tau2 = nrm[:, 2:3], nrm[:, 3:4], nrm[:, 4:5]
            tau4, tau6 = nrm[:, 5:6], nrm[:, 6:7]

            for b in range(BPC):
                # split the input DMA so early chunks can start sooner;
                # finer-grained for the first image (it gates the pipeline)
                if b == 0:
                    slab = slab0
                    cuts = [1400, 3200, 5400, 8000, 10500, SLABF]
                else:
                    slab = spool.tile([128, SLABF], f16, tag="slab")
                    step = (SLABF + NSPLIT - 1) // NSPLIT
                    cuts = [min(k * step, SLABF) for k in range(NSPLIT + 1)]
                for n, (lo, hi) in enumerate(zip(cuts[:-1], cuts[1:])):
                    eng = nc.sync if n % 2 == 0 else nc.scalar
                    eng.dma_start(out=slab[:, lo:hi], in_=xin_t.ap()[b, :, lo:hi])
                slabv = slab[:].rearrange("p (r w) -> p r w", w=WPAD)

                oslab = opool.tile([128, HALF], u8)
                for j in range(NPAIR):
                    ps = ppool.tile([128, NCH], f32)
                    # the two column-tiled halves (adjacent 4-row chunks) are
                    # issued interleaved per tap so the PE runs them
                    # concurrently; the row-structured rhs AP (4 x 112 of the
                    # padded 114-wide slab) skips the seam columns entirely
                    for t in range(NTAPS):
                        dh, dw = divmod(t, 3)
                        for half in range(2):
                            r0 = 4 * (2 * j + half) + dh
                            nc.tensor.matmul(
                                ps[half * Cout:(half + 1) * Cout, :],
                                wts[:, t, :],
                                slabv[:, r0:r0 + 4, dw:dw + W],
                                tile_position=(0, half * Cout),
                                start=(t == 0), stop=(t == NTAPS - 1))
                    z = zpool.tile([128, NCH], f32)
                    nc.scalar.activation(
                        z[:], ps[:], mybir.ActivationFunctionType.Identity,
                        bias=bias, scale=scale)
                    u = upool.tile([128, NCH], mybir.dt.bfloat16)
                    nc.vector._custom_dve(
                        bucket3, out=u[:], in0=z[:],
                        in1=tau2, s0=tau0, s1=tau1)
                    nc.vector._custom_dve(
                        bucket4acc, out=oslab[:, j * NCH:(j + 1) * NCH],
                        in0=z[:], in1=u[:], s0=tau4, s1=tau6)

                # split output DMA so early pieces leave while later chunks
                # are still being computed; finest for the last image (tail)
                ocuts = ([0, HALF // 2, 3 * HALF // 4, 7 * HALF // 8, HALF]
                         if b == BPC - 1 else [0, HALF // 2, HALF])
                for lo, hi in zip(ocuts[:-1], ocuts[1:]):
                    nc.sync.dma_start(out=out_t.ap()[b, :, lo:hi],
                                      in_=oslab[:, lo:hi])

    nc.compile()
    _built.append(nc)
    return nc


def _binarize_weights(w):
    """Exactly the reference's fp32 binarization. Returns (sign in {-1,0,1}, sw)."""
    w = np.asarray(w, np.float32)
    C = w.shape[0]
    wf = w.reshape(C, -1)
    bw = w - wf.mean(-1)[:, None, None, None]
    bw = bw / bw.reshape(C, -1).std(-1, ddof=1)[:, None, None, None]
    mean_abs = np.abs(bw).reshape(C, -1).mean(-1)
    sw = np.exp2(np.round(np.log2(mean_abs))).astype(np.float32)
    return np.sign(bw).astype(np.float32), sw


def kernel(x, w, lut):
    x = np.ascontiguousarray(np.asarray(x, np.float32))
    w = np.asarray(w, np.float32)
    lut = np.asarray(lut, np.float32)

    nc = _build()
    from concourse import bass_utils

    # ---- weights: binarize + fold the pow2 scale into the thresholds ----
    sgn, sw = _binarize_weights(w)                     # sgn [Cout,Cin,3,3]
    t64 = lut.astype(np.float64) / sw[:, None]         # [Cout,7] thresholds

    # lhsT per tap: wts[ci, t, co] = sgn[co, ci, dh, dw]; rows 64-127 (the lo
    # half of K) use the same weights
    wts = np.empty((128, NTAPS, Cout), np.float32)
    for t in range(NTAPS):
        wts[:Cin, t, :] = sgn[:, :, t // 3, t % 3].T
    wts[Cin:] = wts[:Cin]
    wts = wts.astype(np.float16)

    # ---- normalize params: z = y*s + b with tau3 -> 0, tau5 -> 1 ----
    # s>0 always; for degenerate channels (t5 == t3) use a huge power of two
    # so [z > 1] still decides [y > t3] exactly.
    t3, t5 = t64[:, 3], t64[:, 5]
    gap = t5 - t3
    s = np.where(gap > 0, 1.0 / np.where(gap > 0, gap, 1.0), 2.0 ** 100)
    bias = -t3 * s
    taus = (t64[:, [0, 1, 2, 4, 6]] - t3[:, None]) * s[:, None]
    half = np.stack([s, bias, taus[:, 0], taus[:, 1], taus[:, 2],
                     taus[:, 3], taus[:, 4]], axis=1).astype(np.float32)
    nrm = np.empty((128, 7), np.float32)
    nrm[:Cout] = half
    nrm[Cout:] = half

    # ---- hi/lo fp16 split, zero-padded flat slabs ----
    hi = x.astype(np.float16)
    lo = (x - hi.astype(np.float32)).astype(np.float16)
    xin = np.zeros((B, 128, SLABF), np.float16)
    view = xin.reshape(B, 128, HP, WPAD)
    view[:, :Cin, 1:H + 1, 1:W + 1] = hi
    view[:, Cin:, 1:H + 1, 1:W + 1] = lo

    # ---- run on the 8 cores (SPMD, batch-sharded) ----
    wts_np = np.ascontiguousarray(wts)
    nrm_np = np.ascontiguousarray(nrm)
    in_maps = [
        {
            "xin": np.ascontiguousarray(xin[c * BPC:(c + 1) * BPC]),
            "wts": wts_np,
            "nrm": nrm_np,
        }
        for c in range(NCORES)
    ]
    try:
        res = bass_utils.run_bass_kernel_spmd(nc, in_maps,
                                              core_ids=list(range(NCORES)))
    except Exception:
        # transient PJRT/compile hiccups happen occasionally; retry once
        res = bass_utils.run_bass_kernel_spmd(nc, in_maps,
                                              core_ids=list(range(NCORES)))
    global last_results
    last_results = res

    # ---- unshard: interleave the adjacent 4-row chunks ----
    out = np.empty((B, Cout, H, W), np.float32)
    for c in range(NCORES):
        o = res.results[c]["out"]                      # [BPC, 128, HALF] u8
        top = o[:, :Cout, :].reshape(BPC, Cout, NPAIR, 4, W)
        bot = o[:, Cout:, :].reshape(BPC, Cout, NPAIR, 4, W)
        yrows = np.stack([top, bot], axis=3)           # [., ., 14, 2, 4, W]
        out[c * BPC:(c + 1) * BPC] = (
            yrows.reshape(BPC, Cout, H, W).astype(np.float32))
    return out



# revision 3
# speedup vs baseline: 1.0510x; 1.0510x over previous
"""Trainium2 Bass kernel for nn_BasicBlock_1w4a_LUT (binarized 3x3 conv + LUT bucketize).

Data-parallel over batch: 8 NeuronCores x 4 images each; no cross-core
communication. Full inputs in, full output out; shard/unshard on the host.

Host prep:
  - Binarize the weights exactly as the reference does; the result is
    sign(bw)*sw with sw>0 per out-channel (sw is a power of two). sw is folded
    into the LUT thresholds so the device weights are exactly +/-1.
  - The input goes to the device as single fp16 (no hi/lo split): the PE
    multiplies +/-1 * fp16 exactly, and the fp16 input-rounding noise across
    the 576-term sum moves ~4e-4 of outputs across a bucket threshold
    (measured 5.1e-3 full-batch rel err, vs the 2e-2 gate).
  - Each fp16 image is zero-padded into a flat 114x114 slab; a slab tile packs
    TWO images: partitions 0-63 image A, 64-127 image B. The 3x3/pad-1 conv
    becomes 9 shifted dot products over the flat vector.
  - Per-channel affine z = y*s + b chosen so bucketize thresholds map to
    tau3 -> 0 and tau5 -> 1 (frees DVE scalar slots; the DVE op set below
    only has 3 per-partition scalar slots per instruction).

Device, per image pair (one slab):
  - The PE runs in 64x64 tiling mode: 4 independent quadrant tiles, each a
    K=64 matmul stream. Per 448-pixel round, quadrants (0,0)/(0,64) process
    two chunks of image A into PSUM tile P and (64,0)/(64,64) two chunks of
    image B into PSUM tile Q - 4 concurrent streams at 1 column/cycle each,
    the full fp16 PE roofline (the old hi/lo scheme spent half its MACs on
    the lo duplicate).
  - ScalarE applies the per-channel affine out of PSUM; two custom DVE ops
    (registered at import time into concourse's custom-DVE table) compute
    out = sum_k [z > tau_k] over all 7 thresholds in 2 passes, writing u8.
  - A few junk matmuls at kernel start warm the PE HAM clock gate
    (1.2 -> 2.4 GHz) while the first input DMA pieces land; input/output
    DMAs are split so compute starts/finishes without waiting for whole
    images.
"""

import numpy as np

# ---- problem constants (hardcoded per contract) ----
B, Cin, Cout, H, W = 32, 64, 64, 112, 112
NCORES = 8
BPC = B // NCORES          # images per core
NSLAB = BPC // 2           # slabs per core (2 images per slab)
HP = H + 2                 # 114 padded rows
WPAD = W + 2               # 114 padded cols
SLABF = HP * WPAD          # 12996 padded image size
HALF = H * W // 2          # 6272 output pixels per partition-half
NCH = 4 * W                # 448 pixels per chunk = 4 image rows (PSUM <= 2KB)
NRND = H // 8              # 14 rounds per slab (2 chunks per image per round)
NTAPS = 9
NSPLIT = 4                 # input slab DMA pieces per slab (after the first)
NWARM = 13                 # PE warm-up matmuls

_built = []
last_results = None


def _register_dve_ops():
    from concourse.dve_spec import (
        Spec, Src0, Src1, C0, C1, C3, One, Zero, lower,
        _spill_c3_to_src1, _has_src1,
    )
    import concourse.dve_ops as dve_ops
    from concourse.dve_ops import DveOp
    from concourse.dve_uop import DveOpSpec

    def register_op(name, spec):
        if name in dve_ops._SUB_OPCODE_FOR_NAME:
            for op in dve_ops.OPS:
                if op.name == name:
                    return op
            raise RuntimeError(name)
        row = max(dve_ops._SUB_OPCODE_FOR_NAME.values()) + 1
        assert row < 0x20, "custom-DVE opcode rows exhausted"
        shas = {}
        for ver in ("v3", "v4"):
            s = DveOpSpec(name=name, opcode=row, uops=lower(spec, ver=ver),
                          rd1_en=_has_src1(spec))
            shas[ver] = s.sha(ver)
        op = DveOp(name, spec, subdim=False, uops_sha=shas)
        dve_ops.OPS.append(op)
        dve_ops.CUSTOM_DVE_SPECS[name] = spec
        dve_ops._SUB_OPCODE_FOR_NAME[name] = row
        return op

    # u = (z>tau0) + (z>tau1) + (z>tau2);  tau2 rides C3 (spilled to in1 [P,1])
    bucket3 = register_op(
        "BUCKET3_ANT",
        Spec(
            body=_spill_c3_to_src1(((Src0 > C0) + (Src0 > C1)) + (Src0 > C3)),
            reference=lambda in0, in1, s0, s1, imm2: (
                (in0 > s0).astype(np.float32) + (in0 > s1)
                + (in0 > np.asarray(in1, np.float32).reshape(-1, 1))
            ),
        ),
    )
    # out = (z>0) + (z>1) + (z>tau4) + (z>tau6) + u
    bucket4acc = register_op(
        "BUCKET4ACC_ANT",
        Spec(
            body=(((Src0 > Zero) + (Src0 > One))
                  + ((Src0 > C0) + (Src0 > C1))) + Src1,
            reference=lambda in0, in1, s0, s1, imm2: (
                (in0 > 0).astype(np.float32) + (in0 > 1)
                + (in0 > s0) + (in0 > s1) + in1
            ),
        ),
    )
    return bucket3, bucket4acc


def _build():
    """Trace + compile the per-core Bass kernel (once per process)."""
    if _built:
        return _built[0]

    import concourse.bacc as bacc
    import concourse.mybir as mybir
    import concourse.tile as tile

    bucket3, bucket4acc = _register_dve_ops()

    f32, f16, u8 = mybir.dt.float32, mybir.dt.float16, mybir.dt.uint8
    nc = bacc.Bacc("TRN2", target_bir_lowering=False, debug=False,
                   num_devices=NCORES)

    xin_t = nc.dram_tensor("xin", [NSLAB, 128, SLABF], f16, kind="ExternalInput")
    wts_t = nc.dram_tensor("wts", [128, NTAPS, Cout], f16, kind="ExternalInput")
    nrm_t = nc.dram_tensor("nrm", [128, 7], f32, kind="ExternalInput")
    out_t = nc.dram_tensor("out", [BPC, 128, HALF], u8, kind="ExternalOutput")

    with tile.TileContext(nc) as tc:
        with (
            tc.tile_pool(name="const", bufs=1) as cpool,
            tc.tile_pool(name="slab", bufs=2) as spool,
            tc.tile_pool(name="psum", bufs=3, space="PSUM") as ppool,
            tc.tile_pool(name="wpsum", bufs=1, space="PSUM") as wpool,
            tc.tile_pool(name="z", bufs=4) as zpool,
            tc.tile_pool(name="u", bufs=4) as upool,
            tc.tile_pool(name="o", bufs=4) as opool,
        ):
            # first slab piece of slab 0 goes out before anything else — it
            # gates the first matmuls
            slab0 = spool.tile([128, SLABF], f16, tag="slab")
            nc.sync.dma_start(out=slab0[:, 0:1400], in_=xin_t.ap()[0, :, 0:1400])
            wts = cpool.tile([128, NTAPS, Cout], f16)
            nc.scalar.dma_start(out=wts[:], in_=wts_t.ap())
            nrm = cpool.tile([128, 7], f32)
            nc.scalar.dma_start(out=nrm[:], in_=nrm_t.ap())

            # PE warm-up: junk matmuls on a zeroed tile while the first input
            # DMAs land, so the HAM clock gate opens (1.2 -> 2.4 GHz) before
            # the real matmuls start. Same 64x64 tile mode as the real
            # matmuls so the mode switch doesn't drain mid-stream.
            wu = cpool.tile([128, NCH], f16)
            nc.gpsimd.memset(wu[:], 0.0)
            wps = wpool.tile([64, NCH], f32)
            for _ in range(NWARM):
                nc.tensor.matmul(wps[:], wu[0:64, 0:Cout], wu[0:64, :],
                                 tile_position=(0, 0), start=True, stop=True)

            scale, bias = nrm[:, 0:1], nrm[:, 1:2]
            tau0, tau1, tau2 = nrm[:, 2:3], nrm[:, 3:4], nrm[:, 4:5]
            tau4, tau6 = nrm[:, 5:6], nrm[:, 6:7]

            for s in range(NSLAB):
                # split the input DMA so early chunks can start sooner;
                # finer-grained for the first slab (it gates the pipeline)
                if s == 0:
                    slab = slab0
                    cuts = [1400, 3200, 5400, 8000, 10500, SLABF]
                else:
                    slab = spool.tile([128, SLABF], f16, tag="slab")
                    step = (SLABF + NSPLIT - 1) // NSPLIT
                    cuts = [min(k * step, SLABF) for k in range(NSPLIT + 1)]
                for n, (lo, hi) in enumerate(zip(cuts[:-1], cuts[1:])):
                    eng = nc.sync if n % 2 == 0 else nc.scalar
                    eng.dma_start(out=slab[:, lo:hi], in_=xin_t.ap()[s, :, lo:hi])
                slabv = slab[:].rearrange("p (r w) -> p r w", w=WPAD)

                oslabA = opool.tile([128, HALF], u8)
                oslabB = opool.tile([128, HALF], u8)
                for j in range(NRND):
                    psP = ppool.tile([128, NCH], f32)
                    psQ = ppool.tile([128, NCH], f32)
                    # 4 independent 64x64 quadrant streams per tap: chunks
                    # (2j, 2j+1) of image A (slab parts 0-63) and of image B
                    # (parts 64-127). The row-structured rhs AP (4 x 112 of
                    # the padded 114-wide slab) skips the seam columns.
                    for t in range(NTAPS):
                        dh, dw = divmod(t, 3)
                        r0, r1 = 8 * j + dh, 8 * j + 4 + dh
                        st, sp = (t == 0), (t == NTAPS - 1)
                        nc.tensor.matmul(
                            psP[0:Cout, :], wts[0:64, t, :],
                            slabv[0:64, r0:r0 + 4, dw:dw + W],
                            tile_position=(0, 0), start=st, stop=sp)
                        nc.tensor.matmul(
                            psQ[0:Cout, :], wts[64:128, t, :],
                            slabv[64:128, r0:r0 + 4, dw:dw + W],
                            tile_position=(64, 0), start=st, stop=sp)
                        nc.tensor.matmul(
                            psP[Cout:128, :], wts[0:64, t, :],
                            slabv[0:64, r1:r1 + 4, dw:dw + W],
                            tile_position=(0, 64), start=st, stop=sp)
                        nc.tensor.matmul(
                            psQ[Cout:128, :], wts[64:128, t, :],
                            slabv[64:128, r1:r1 + 4, dw:dw + W],
                            tile_position=(64, 64), start=st, stop=sp)
                    for ps, oslab in ((psP, oslabA), (psQ, oslabB)):
                        z = zpool.tile([128, NCH], f32)
                        nc.scalar.activation(
                            z[:], ps[:], mybir.ActivationFunctionType.Identity,
                            bias=bias, scale=scale)
                        u = upool.tile([128, NCH], mybir.dt.bfloat16)
                        nc.vector._custom_dve(
                            bucket3, out=u[:], in0=z[:],
                            in1=tau2, s0=tau0, s1=tau1)
                        nc.vector._custom_dve(
                            bucket4acc, out=oslab[:, j * NCH:(j + 1) * NCH],
                            in0=z[:], in1=u[:], s0=tau4, s1=tau6)

                # split output DMA so early pieces leave while later rounds
                # are still being computed; finest for the last slab (tail)
                ocuts = ([0, HALF // 2, 3 * HALF // 4, 7 * HALF // 8, HALF]
                         if s == NSLAB - 1 else [0, HALF // 2, HALF])
                for lo, hi in zip(ocuts[:-1], ocuts[1:]):
                    nc.sync.dma_start(out=out_t.ap()[2 * s, :, lo:hi],
                                      in_=oslabA[:, lo:hi])
                    nc.sync.dma_start(out=out_t.ap()[2 * s + 1, :, lo:hi],
                                      in_=oslabB[:, lo:hi])

    nc.compile()
    _built.append(nc)
    return nc


def _binarize_weights(w):
    """Exactly the reference's fp32 binarization. Returns (sign in {-1,0,1}, sw)."""
    w = np.asarray(w, np.float32)
    C = w.shape[0]
    wf = w.reshape(C, -1)
    bw = w - wf.mean(-1)[:, None, None, None]
    bw = bw / bw.reshape(C, -1).std(-1, ddof=1)[:, None, None, None]
    mean_abs = np.abs(bw).reshape(C, -1).mean(-1)
    sw = np.exp2(np.round(np.log2(mean_abs))).astype(np.float32)
    return np.sign(bw).astype(np.float32), sw


def kernel(x, w, lut):
    x = np.ascontiguousarray(np.asarray(x, np.float32))
    w = np.asarray(w, np.float32)
    lut = np.asarray(lut, np.float32)

    nc = _build()
    from concourse import bass_utils

    # ---- weights: binarize + fold the pow2 scale into the thresholds ----
    sgn, sw = _binarize_weights(w)                     # sgn [Cout,Cin,3,3]
    t64 = lut.astype(np.float64) / sw[:, None]         # [Cout,7] thresholds

    # lhsT per tap: wts[ci, t, co] = sgn[co, ci, dh, dw]; rows 64-127 (the
    # image-B row tiles) use the same weights
    wts = np.empty((128, NTAPS, Cout), np.float32)
    for t in range(NTAPS):
        wts[:Cin, t, :] = sgn[:, :, t // 3, t % 3].T
    wts[Cin:] = wts[:Cin]
    wts = wts.astype(np.float16)

    # ---- normalize params: z = y*s + b with tau3 -> 0, tau5 -> 1 ----
    # s>0 always; for degenerate channels (t5 == t3) use a huge power of two
    # so [z > 1] still decides [y > t3] exactly.
    t3, t5 = t64[:, 3], t64[:, 5]
    gap = t5 - t3
    s = np.where(gap > 0, 1.0 / np.where(gap > 0, gap, 1.0), 2.0 ** 100)
    bias = -t3 * s
    taus = (t64[:, [0, 1, 2, 4, 6]] - t3[:, None]) * s[:, None]
    half = np.stack([s, bias, taus[:, 0], taus[:, 1], taus[:, 2],
                     taus[:, 3], taus[:, 4]], axis=1).astype(np.float32)
    nrm = np.empty((128, 7), np.float32)
    nrm[:Cout] = half
    nrm[Cout:] = half

    # ---- single-fp16 images, zero-padded flat slabs, 2 images per slab ----
    hi = x.astype(np.float16)                          # [B, Cin, H, W]
    xin = np.zeros((B // 2, 128, SLABF), np.float16)
    view = xin.reshape(B // 2, 128, HP, WPAD)
    view[:, :Cin, 1:H + 1, 1:W + 1] = hi[0::2]
    view[:, Cin:, 1:H + 1, 1:W + 1] = hi[1::2]

    # ---- run on the 8 cores (SPMD, batch-sharded) ----
    wts_np = np.ascontiguousarray(wts)
    nrm_np = np.ascontiguousarray(nrm)
    in_maps = [
        {
            "xin": np.ascontiguousarray(xin[c * NSLAB:(c + 1) * NSLAB]),
            "wts": wts_np,
            "nrm": nrm_np,
        }
        for c in range(NCORES)
    ]
    try:
        res = bass_utils.run_bass_kernel_spmd(nc, in_maps,
                                              core_ids=list(range(NCORES)))
    except Exception:
        # transient PJRT/compile hiccups happen occasionally; retry once
        res = bass_utils.run_bass_kernel_spmd(nc, in_maps,
                                              core_ids=list(range(NCORES)))
    global last_results
    last_results = res

    # ---- unshard: interleave the adjacent 4-row chunks ----
    out = np.empty((B, Cout, H, W), np.float32)
    for c in range(NCORES):
        o = res.results[c]["out"]                      # [BPC, 128, HALF] u8
        top = o[:, :Cout, :].reshape(BPC, Cout, NRND, 4, W)
        bot = o[:, Cout:, :].reshape(BPC, Cout, NRND, 4, W)
        yrows = np.stack([top, bot], axis=3)           # [., ., 14, 2, 4, W]
        out[c * BPC:(c + 1) * BPC] = (
            yrows.reshape(BPC, Cout, H, W).astype(np.float32))
    return out


# revision 12
# speedup vs baseline: 1.5882x; 1.5112x over previous
"""Trainium2 Bass kernel for nn_BasicBlock_1w4a_LUT (binarized 3x3 conv + LUT bucketize).

Data-parallel over batch: 8 NeuronCores x 4 images each; no cross-core
communication. Full inputs in, full output out; shard/unshard on the host.

Host prep:
  - Binarize the weights exactly as the reference does; sign(bw)*sw with sw a
    power of two per out-channel, folded into the thresholds so the device
    weights are exactly +/-1.
  - The input goes to the device as single fp16 (no hi/lo split): the PE
    multiplies +/-1 * fp16 exactly; fp16 input-rounding noise across the
    576-term sum moves ~4e-4 of outputs across a bucket threshold (measured
    5e-3 full-batch rel err vs the 2e-2 gate).
  - Each fp16 image is zero-padded into a flat 114x114 slab; a slab tile
    packs TWO images: partitions 0-63 image A, 64-127 image B.
  - The LUT is round(u + k*d) per channel (BN-folded arithmetic sequence), so
    bucketize(y) = clamp(ceil((ceil(y) - u')/d), 0, 7). (u', d) are recovered
    per channel by a max-margin 1D fit (margins ~0.2 >> fp32 noise).

Device, per image pair (one slab):
  - PE in 64x64 tiling mode: 4 independent K=64 quadrant streams. Rounds are
    processed in pairs with a joint tap loop so each quadrant runs 2
    back-to-back matmuls with the same weights (hides the LDWEIGHTS bubble).
    Per round one [128,1024] PSUM tile (2 banks): cols 0:448 image A (chunk
    2j top / 2j+1 bottom), cols 512:960 image B.
  - ScalarE: one activation per round, z = y + 0.5 (PSUM -> SBUF f32).
  - VectorE: ONE custom 8-stage DVE op per round computes the whole 7-level
    bucketize via two magic-number roundings (B = 1.5*2^23):
      out = max(min(rne(rne(z+B)-B)*r + D')+B, HI) - B, 0)   [rne via +B]
    writing u8. Junk cols 448:512 are skipped by the strided output DMA.
  - Junk matmuls at kernel start warm the PE HAM clock gate (1.2 -> 2.4 GHz)
    while the first input DMA pieces land; input/output DMAs are split so
    compute starts/finishes without waiting for whole images.
"""

import numpy as np

# ---- problem constants (hardcoded per contract) ----
B, Cin, Cout, H, W = 32, 64, 64, 112, 112
NCORES = 8
BPC = B // NCORES          # images per core
NSLAB = BPC // 2           # slabs per core (2 images per slab)
HP = H + 2                 # 114 padded rows
WPAD = W + 2               # 114 padded cols
SLABF = HP * WPAD          # 12996 padded image size
HALF = H * W // 2          # 6272 output pixels per partition-half per image
NCH = 4 * W                # 448 pixels per chunk = 4 image rows
PSW = 1024                 # psum tile width (2 banks; B at 0:448, junk, A at 512:960)
NRND = H // 8              # 14 rounds per slab (2 chunks per image per round)
NTAPS = 9
NWARM = 13                 # PE warm-up matmuls
MAGIC = np.float32(1.5 * 2 ** 23)

_built = {}
last_results = None


def _register_dve_ops():
    from concourse.dve_spec import (
        Spec, Src0, Src1, C0, C1, C2, C3, Zero, lower,
        _spill_c3_to_src1, _has_src1, maxx, minn,
    )
    import concourse.dve_ops as dve_ops
    from concourse.dve_ops import DveOp
    from concourse.dve_uop import DveOpSpec

    def register_op(name, spec):
        if name in dve_ops._SUB_OPCODE_FOR_NAME:
            for op in dve_ops.OPS:
                if op.name == name:
                    return op
            raise RuntimeError(name)
        row = max(dve_ops._SUB_OPCODE_FOR_NAME.values()) + 1
        assert row < 0x20, "custom-DVE opcode rows exhausted"
        shas = {}
        for ver in ("v3", "v4"):
            s = DveOpSpec(name=name, opcode=row, uops=lower(spec, ver=ver),
                          rd1_en=_has_src1(spec))
            shas[ver] = s.sha(ver)
        op = DveOp(name, spec, subdim=False, uops_sha=shas)
        dve_ops.OPS.append(op)
        dve_ops.CUSTOM_DVE_SPECS[name] = spec
        dve_ops._SUB_OPCODE_FOR_NAME[name] = row
        return op

    # Full 7-threshold bucketize in one op. in0 = z = y + 0.5 (f32),
    # s0 = r = 1/d, s1 = D' = 0.5 - u'*r, imm2 = B (magic), in1 = HI = B + 7.
    #   t1 = z + B            -> ceil(y) + B   (rne on the ulp-1 grid)
    #   c  = t1 - B           -> ceil(y), exact
    #   t4 = c*r + D'         -> v + 0.5,  v = (c - u')/d
    #   t5 = t4 + B           -> ceil(v) + B
    #   out = max(min(t5, HI) - B, 0)  in {0..7}, written as u8
    stair = register_op(
        "STAIR8_ANT",
        Spec(
            body=_spill_c3_to_src1(
                maxx(minn((((((Src0 + C2) - C2) * C0) + C1) + C2), C3) - C2,
                     Zero)),
            reference=lambda in0, in1, s0, s1, imm2: np.maximum(
                np.minimum(
                    ((np.float32(in0 + np.float32(imm2))
                      - np.float32(imm2))
                     * np.asarray(s0, np.float32).reshape(-1, 1)
                     + np.asarray(s1, np.float32).reshape(-1, 1))
                    + np.float32(imm2),
                    np.asarray(in1, np.float32).reshape(-1, 1))
                - np.float32(imm2), np.float32(0)),
        ),
    )
    return stair


def _build():
    """Trace + compile the per-core Bass kernel (once per process)."""
    if "nc" in _built:
        return _built["nc"]

    import concourse.bacc as bacc
    import concourse.mybir as mybir
    import concourse.tile as tile

    stair = _register_dve_ops()

    f32, f16, u8 = mybir.dt.float32, mybir.dt.float16, mybir.dt.uint8
    nc = bacc.Bacc("TRN2", target_bir_lowering=False, debug=False,
                   num_devices=NCORES)

    xin_t = nc.dram_tensor("xin", [NSLAB, 128, SLABF], f16, kind="ExternalInput")
    wts_t = nc.dram_tensor("wts", [128, NTAPS, Cout], f16, kind="ExternalInput")
    nrm_t = nc.dram_tensor("nrm", [128, 5], f32, kind="ExternalInput")
    out_t = nc.dram_tensor("out", [BPC, 128, HALF], u8, kind="ExternalOutput")

    with tile.TileContext(nc) as tc:
        with (
            tc.tile_pool(name="const", bufs=1) as cpool,
            tc.tile_pool(name="slab", bufs=2) as spool,
            tc.tile_pool(name="psum", bufs=2, space="PSUM") as ppool,
            tc.tile_pool(name="z", bufs=3) as zpool,
            tc.tile_pool(name="o", bufs=2) as opool,
        ):
            # first slab piece of slab 0 goes out before anything else — it
            # gates the first matmuls
            slab0 = spool.tile([128, SLABF], f16, tag="slab")
            nc.sync.dma_start(out=slab0[:, 0:1400], in_=xin_t.ap()[0, :, 0:1400])
            wts = cpool.tile([128, NTAPS, Cout], f16)
            nc.scalar.dma_start(out=wts[:], in_=wts_t.ap())
            nrm = cpool.tile([128, 5], f32)
            nc.scalar.dma_start(out=nrm[:], in_=nrm_t.ap())

            # PE warm-up: junk matmuls on a zeroed tile while the first input
            # DMAs land, so the HAM clock gate opens (1.2 -> 2.4 GHz) before
            # the real matmuls start. Same 64x64 tile mode as the real ones;
            # they scribble on the first pair's PSUM tile, which the first
            # real accumulation group (start=True) overwrites.
            wu = cpool.tile([128, NCH], f16)
            nc.gpsimd.memset(wu[:], 0.0)
            ps00 = [ppool.tile([128, PSW], f32, name="psA"),
                    ppool.tile([128, PSW], f32, name="psB")]
            for _ in range(NWARM):
                nc.tensor.matmul(ps00[0][0:64, 0:NCH], wu[0:64, 0:Cout],
                                 wu[0:64, :], tile_position=(0, 0),
                                 start=True, stop=True)

            rconst, dconst = nrm[:, 0:1], nrm[:, 1:2]
            hiconst, bias05, one = nrm[:, 2:3], nrm[:, 3:4], nrm[:, 4:5]

            for s in range(NSLAB):
                # split the input DMA so early chunks can start sooner;
                # finer-grained for the first slab (it gates the pipeline)
                if s == 0:
                    slab = slab0
                    cuts = [1400, 3200, 5400, 8000, 10500, SLABF]
                else:
                    slab = spool.tile([128, SLABF], f16, tag="slab")
                    cuts = [0, 3300, 6600, 9900, SLABF]
                for n, (lo, hi) in enumerate(zip(cuts[:-1], cuts[1:])):
                    eng = nc.sync if n % 2 == 0 else nc.scalar
                    eng.dma_start(out=slab[:, lo:hi], in_=xin_t.ap()[s, :, lo:hi])
                slabv = slab[:].rearrange("p (r w) -> p r w", w=WPAD)

                oslab = opool.tile([128, NRND * PSW], u8)
                # rounds in pairs: joint tap loop over 2 rounds so each
                # quadrant tile runs 2 consecutive matmuls with the same
                # weights (LDWEIGHTS for the next tap hides under the 2nd)
                for jj in range(NRND // 2):
                    if s == 0 and jj == 0:
                        ps = ps00
                    else:
                        ps = [ppool.tile([128, PSW], f32, name="psA"),
                              ppool.tile([128, PSW], f32, name="psB")]
                    for t in range(NTAPS):
                        dh, dw = divmod(t, 3)
                        st, sp = (t == 0), (t == NTAPS - 1)
                        for half in range(2):      # chunk within image
                            for g in range(2):     # round within pair
                                j = 2 * jj + g
                                r0 = 8 * j + 4 * half + dh
                                rhsA = slabv[0:64, r0:r0 + 4, dw:dw + W]
                                rhsB = slabv[64:128, r0:r0 + 4, dw:dw + W]
                                po = half * Cout
                                # image A -> cols 0:448, image B -> 512:960
                                nc.tensor.matmul(
                                    ps[g][po:po + Cout, 0:NCH],
                                    wts[0:64, t, :], rhsA,
                                    tile_position=(0, po), start=st, stop=sp)
                                nc.tensor.matmul(
                                    ps[g][po:po + Cout, 512:512 + NCH],
                                    wts[64:128, t, :], rhsB,
                                    tile_position=(64, po), start=st, stop=sp)
                    for g in range(2):
                        j = 2 * jj + g
                        z = zpool.tile([128, 960], f32)
                        nc.scalar.activation(
                            z[:], ps[g][:, 0:960],
                            mybir.ActivationFunctionType.Identity,
                            bias=bias05, scale=one)
                        nc.vector._custom_dve(
                            stair, out=oslab[:, j * PSW:j * PSW + 960],
                            in0=z[:], in1=hiconst, s0=rconst, s1=dconst,
                            imm2=float(MAGIC))

                # strided output DMA: image A = col block [0:448] of each
                # round, image B = [512:960]; junk cols never leave SBUF
                ov = oslab[:].rearrange("p (j c) -> p j c", c=PSW)
                jcuts = [0, 4, 7, 10, NRND] if s == NSLAB - 1 else [0, 7, NRND]
                for lo, hi in zip(jcuts[:-1], jcuts[1:]):
                    nc.sync.dma_start(
                        out=out_t.ap()[2 * s, :, lo * NCH:hi * NCH],
                        in_=ov[:, lo:hi, 0:NCH])
                    nc.sync.dma_start(
                        out=out_t.ap()[2 * s + 1, :, lo * NCH:hi * NCH],
                        in_=ov[:, lo:hi, 512:512 + NCH])

    nc.compile()
    _built["nc"] = nc
    return nc


def _binarize_weights(w):
    """Exactly the reference's fp32 binarization. Returns (sign in {-1,0,1}, sw)."""
    w = np.asarray(w, np.float32)
    C = w.shape[0]
    wf = w.reshape(C, -1)
    bw = w - wf.mean(-1)[:, None, None, None]
    bw = bw / bw.reshape(C, -1).std(-1, ddof=1)[:, None, None, None]
    mean_abs = np.abs(bw).reshape(C, -1).mean(-1)
    sw = np.exp2(np.round(np.log2(mean_abs))).astype(np.float32)
    return np.sign(bw).astype(np.float32), sw


def _fit_lattice(t64):
    """Per channel find (u', d): lut_k in (u'+k*d-1, u'+k*d) ... i.e.
    lut_k < u' + k*d < lut_k + 1 for k=0..6, maximizing the margin.
    The LUT is round(u + k*d) of an arithmetic sequence, so this is feasible
    with margin ~0.25 generically. Returns (r, Dp, ok)."""
    k = np.arange(7)
    C = t64.shape[0]
    dd = np.arange(0.25, 4.0, 5e-5)                    # [M]
    lo = (t64[:, None, :] - k[None, None, :] * dd[None, :, None]).max(-1)
    hi = (t64[:, None, :] + 1 - k[None, None, :] * dd[None, :, None]).min(-1)
    marg = hi - lo                                      # [C, M]
    best = marg.argmax(1)
    m = marg[np.arange(C), best]
    d = dd[best]
    u = (lo[np.arange(C), best] + hi[np.arange(C), best]) / 2
    r = (1.0 / d).astype(np.float32)
    Dp = (0.5 - u / d).astype(np.float32)
    return r, Dp, m.min()


def kernel(x, w, lut):
    x = np.ascontiguousarray(np.asarray(x, np.float32))
    w = np.asarray(w, np.float32)
    lut = np.asarray(lut, np.float32)

    nc = _build()
    from concourse import bass_utils

    # ---- weights: binarize + fold the pow2 scale into the thresholds ----
    sgn, sw = _binarize_weights(w)                     # sgn [Cout,Cin,3,3]
    t64 = lut.astype(np.float64) / sw[:, None]         # [Cout,7] thresholds

    # The staircase formulation needs integer thresholds: t64 = lut/sw with
    # lut integer and sw a power of two. sw != 1 would make them non-integer;
    # verify (sw == 1 for the reference's BN stats; the conv scale then
    # cancels exactly).
    assert np.all(sw == 1.0), f"unexpected weight scale {np.unique(sw)}"
    assert np.all(t64 == np.round(t64)), "thresholds not integers"

    # lhsT per tap: wts[ci, t, co] = sgn[co, ci, dh, dw]; rows 64-127 (the
    # image-B row tiles) use the same weights
    wts = np.empty((128, NTAPS, Cout), np.float32)
    for t in range(NTAPS):
        wts[:Cin, t, :] = sgn[:, :, t // 3, t % 3].T
    wts[Cin:] = wts[:Cin]
    wts = wts.astype(np.float16)

    # ---- staircase params ----
    r, Dp, min_margin = _fit_lattice(t64)
    assert min_margin > 1e-3, f"lattice fit failed (margin {min_margin})"
    half = np.stack([r, Dp,
                     np.full(Cout, MAGIC + 7, np.float32),
                     np.full(Cout, 0.5, np.float32),
                     np.full(Cout, 1.0, np.float32)], axis=1)
    nrm = np.empty((128, 5), np.float32)
    nrm[:Cout] = half
    nrm[Cout:] = half

    # ---- single-fp16 images, zero-padded flat slabs, 2 images per slab ----
    hi = x.astype(np.float16)                          # [B, Cin, H, W]
    xin = np.zeros((B // 2, 128, SLABF), np.float16)
    view = xin.reshape(B // 2, 128, HP, WPAD)
    view[:, :Cin, 1:H + 1, 1:W + 1] = hi[0::2]
    view[:, Cin:, 1:H + 1, 1:W + 1] = hi[1::2]

    # ---- run on the 8 cores (SPMD, batch-sharded) ----
    wts_np = np.ascontiguousarray(wts)
    nrm_np = np.ascontiguousarray(nrm)
    in_maps = [
        {
            "xin": np.ascontiguousarray(xin[c * NSLAB:(c + 1) * NSLAB]),
            "wts": wts_np,
            "nrm": nrm_np,
        }
        for c in range(NCORES)
    ]
    try:
        res = bass_utils.run_bass_kernel_spmd(nc, in_maps,
                                              core_ids=list(range(NCORES)))
    except Exception:
        # transient PJRT/compile hiccups happen occasionally; retry once
        res = bass_utils.run_bass_kernel_spmd(nc, in_maps,
                                              core_ids=list(range(NCORES)))
    global last_results
    last_results = res

    # ---- unshard: interleave the adjacent 4-row chunks ----
    out = np.empty((B, Cout, H, W), np.float32)
    for c in range(NCORES):
        o = res.results[c]["out"]                      # [BPC, 128, HALF] u8
        top = o[:, :Cout, :].reshape(BPC, Cout, NRND, 4, W)
        bot = o[:, Cout:, :].reshape(BPC, Cout, NRND, 4, W)
        yrows = np.stack([top, bot], axis=3)           # [., ., 14, 2, 4, W]
        out[c * BPC:(c + 1) * BPC] = (
            yrows.reshape(BPC, Cout, H, W).astype(np.float32))
    return out


# revision 18
# speedup vs baseline: 1.6342x; 1.0290x over previous
"""Trainium2 Bass kernel for nn_BasicBlock_1w4a_LUT (binarized 3x3 conv + LUT bucketize).

Data-parallel over batch: 8 NeuronCores x 4 images each; no cross-core
communication. Full inputs in, full output out; shard/unshard on the host.

Host prep:
  - Binarize the weights exactly as the reference does; sign(bw)*sw with sw a
    power of two per out-channel, folded into the thresholds so the device
    weights are exactly +/-1.
  - The input goes to the device as single fp16 (no hi/lo split): the PE
    multiplies +/-1 * fp16 exactly; fp16 input-rounding noise across the
    576-term sum moves ~4e-4 of outputs across a bucket threshold (measured
    5e-3 full-batch rel err vs the 2e-2 gate).
  - Each fp16 image is zero-padded into a flat 114x114 slab; a slab tile
    packs TWO images: partitions 0-63 image A, 64-127 image B.
  - The LUT is round(u + k*d) per channel (BN-folded arithmetic sequence), so
    bucketize(y) = clamp(ceil((ceil(y) - u')/d), 0, 7). (u', d) are recovered
    per channel by a max-margin 1D fit (margins ~0.2 >> fp32 noise).

Device, per image pair (one slab):
  - PE in 64x64 tiling mode: 4 independent K=64 quadrant streams. Rounds are
    processed in pairs with a joint tap loop so each quadrant runs 2
    back-to-back matmuls with the same weights (hides the LDWEIGHTS bubble).
    Per round one [128,1024] PSUM tile (2 banks): cols 0:448 image A (chunk
    2j top / 2j+1 bottom), cols 512:960 image B.
  - ScalarE: one activation per round, z = y + 0.5 (PSUM -> SBUF f32).
  - VectorE: ONE custom 8-stage DVE op per round computes the whole 7-level
    bucketize via two magic-number roundings (B = 1.5*2^23):
      out = max(min(rne(rne(z+B)-B)*r + D')+B, HI) - B, 0)   [rne via +B]
    writing u8. Junk cols 448:512 are skipped by the strided output DMA.
  - Junk matmuls at kernel start warm the PE HAM clock gate (1.2 -> 2.4 GHz)
    while the first input DMA pieces land; input/output DMAs are split so
    compute starts/finishes without waiting for whole images.
"""

import numpy as np

# ---- problem constants (hardcoded per contract) ----
B, Cin, Cout, H, W = 32, 64, 64, 112, 112
NCORES = 8
BPC = B // NCORES          # images per core
NSLAB = BPC // 2           # slabs per core (2 images per slab)
HP = H + 2                 # 114 padded rows
WPAD = W + 2               # 114 padded cols
SLABF = HP * WPAD          # 12996 padded image size
HALF = H * W // 2          # 6272 output pixels per partition-half per image
NCH = 4 * W                # 448 pixels per chunk = 4 image rows
PSW = 1024                 # psum tile width (2 banks; B at 0:448, junk, A at 512:960)
NRND = H // 8              # 14 rounds per slab (2 chunks per image per round)
NTAPS = 9
NWARM = 8                  # PE warm-up matmuls
MAGIC = np.float32(1.5 * 2 ** 23)

_built = {}
last_results = None


def _register_dve_ops():
    from concourse.dve_spec import (
        Spec, Src0, Src1, C0, C1, C2, C3, Zero, lower,
        _spill_c3_to_src1, _has_src1, maxx, minn,
    )
    import concourse.dve_ops as dve_ops
    from concourse.dve_ops import DveOp
    from concourse.dve_uop import DveOpSpec

    def register_op(name, spec):
        if name in dve_ops._SUB_OPCODE_FOR_NAME:
            for op in dve_ops.OPS:
                if op.name == name:
                    return op
            raise RuntimeError(name)
        row = max(dve_ops._SUB_OPCODE_FOR_NAME.values()) + 1
        assert row < 0x20, "custom-DVE opcode rows exhausted"
        shas = {}
        for ver in ("v3", "v4"):
            s = DveOpSpec(name=name, opcode=row, uops=lower(spec, ver=ver),
                          rd1_en=_has_src1(spec))
            shas[ver] = s.sha(ver)
        op = DveOp(name, spec, subdim=False, uops_sha=shas)
        dve_ops.OPS.append(op)
        dve_ops.CUSTOM_DVE_SPECS[name] = spec
        dve_ops._SUB_OPCODE_FOR_NAME[name] = row
        return op

    # Full 7-threshold bucketize in one op. in0 = z = y + 0.5 (f32),
    # s0 = r = 1/d, s1 = D' = 0.5 - u'*r, imm2 = B (magic), in1 = HI = B + 7.
    #   t1 = z + B            -> ceil(y) + B   (rne on the ulp-1 grid)
    #   c  = t1 - B           -> ceil(y), exact
    #   t4 = c*r + D'         -> v + 0.5,  v = (c - u')/d
    #   t5 = t4 + B           -> ceil(v) + B
    #   out = max(min(t5, HI) - B, 0)  in {0..7}, written as u8
    stair = register_op(
        "STAIR8_ANT",
        Spec(
            body=_spill_c3_to_src1(
                maxx(minn((((((Src0 + C2) - C2) * C0) + C1) + C2), C3) - C2,
                     Zero)),
            reference=lambda in0, in1, s0, s1, imm2: np.maximum(
                np.minimum(
                    ((np.float32(in0 + np.float32(imm2))
                      - np.float32(imm2))
                     * np.asarray(s0, np.float32).reshape(-1, 1)
                     + np.asarray(s1, np.float32).reshape(-1, 1))
                    + np.float32(imm2),
                    np.asarray(in1, np.float32).reshape(-1, 1))
                - np.float32(imm2), np.float32(0)),
        ),
    )
    return stair


def _build():
    """Trace + compile the per-core Bass kernel (once per process)."""
    if "nc" in _built:
        return _built["nc"]

    import concourse.bacc as bacc
    import concourse.mybir as mybir
    import concourse.tile as tile

    stair = _register_dve_ops()

    f32, f16, u8 = mybir.dt.float32, mybir.dt.float16, mybir.dt.uint8
    nc = bacc.Bacc("TRN2", target_bir_lowering=False, debug=False,
                   num_devices=NCORES)

    xin_t = nc.dram_tensor("xin", [NSLAB, 128, SLABF], f16, kind="ExternalInput")
    wts_t = nc.dram_tensor("wts", [128, NTAPS, Cout], f16, kind="ExternalInput")
    nrm_t = nc.dram_tensor("nrm", [128, 5], f32, kind="ExternalInput")
    out_t = nc.dram_tensor("out", [BPC, 128, HALF], u8, kind="ExternalOutput")

    with tile.TileContext(nc) as tc:
        with (
            tc.tile_pool(name="const", bufs=1) as cpool,
            tc.tile_pool(name="slab", bufs=2) as spool,
            tc.tile_pool(name="psum", bufs=2, space="PSUM") as ppool,
            tc.tile_pool(name="z", bufs=3) as zpool,
            tc.tile_pool(name="o", bufs=2) as opool,
        ):
            # first slab piece of slab 0 goes out before anything else — it
            # gates the first matmuls
            slab0 = spool.tile([128, SLABF], f16, tag="slab")
            nc.sync.dma_start(out=slab0[:, 0:2100], in_=xin_t.ap()[0, :, 0:2100])
            wts = cpool.tile([128, NTAPS, Cout], f16)
            nc.scalar.dma_start(out=wts[:], in_=wts_t.ap())
            nrm = cpool.tile([128, 5], f32)
            nc.scalar.dma_start(out=nrm[:], in_=nrm_t.ap())

            # PE warm-up: junk matmuls on a zeroed tile while the first input
            # DMAs land, so the HAM clock gate opens (1.2 -> 2.4 GHz) before
            # the real matmuls start (memset on VectorE so the warm-up isn't
            # gated on anything slow). Same 64x64 tile mode as the real ones;
            # they scribble on the first pair's PSUM tile, which the first
            # real accumulation group (start=True) overwrites.
            wu = cpool.tile([128, NCH], f16)
            nc.vector.memset(wu[:], 0.0)
            ps00 = [ppool.tile([128, PSW], f32, name="psA"),
                    ppool.tile([128, PSW], f32, name="psB")]
            for _ in range(NWARM):
                nc.tensor.matmul(ps00[0][0:64, 0:NCH], wu[0:64, 0:Cout],
                                 wu[0:64, :], tile_position=(0, 0),
                                 start=True, stop=True)

            rconst, dconst = nrm[:, 0:1], nrm[:, 1:2]
            hiconst, bias05, one = nrm[:, 2:3], nrm[:, 3:4], nrm[:, 4:5]

            for s in range(NSLAB):
                # split the input DMA so early chunks can start sooner;
                # finer-grained for the first slab (it gates the pipeline)
                if s == 0:
                    slab = slab0
                    cuts = [2100, 4000, 6000, 8200, 10600, SLABF]
                else:
                    slab = spool.tile([128, SLABF], f16, tag="slab")
                    cuts = [0, 3300, 6600, 9900, SLABF]
                for n, (lo, hi) in enumerate(zip(cuts[:-1], cuts[1:])):
                    eng = nc.sync if n % 2 == 0 else nc.scalar
                    eng.dma_start(out=slab[:, lo:hi], in_=xin_t.ap()[s, :, lo:hi])
                slabv = slab[:].rearrange("p (r w) -> p r w", w=WPAD)

                oslab = opool.tile([128, NRND * PSW], u8)
                # rounds in pairs: joint tap loop over 2 rounds so each
                # quadrant tile runs 2 consecutive matmuls with the same
                # weights (LDWEIGHTS for the next tap hides under the 2nd)
                for jj in range(NRND // 2):
                    if s == 0 and jj == 0:
                        ps = ps00
                    else:
                        ps = [ppool.tile([128, PSW], f32, name="psA"),
                              ppool.tile([128, PSW], f32, name="psB")]
                    def mm_tap(g, t, st, sp):
                        dh, dw = divmod(t, 3)
                        for half in range(2):      # chunk within image
                            j = 2 * jj + g
                            r0 = 8 * j + 4 * half + dh
                            rhsA = slabv[0:64, r0:r0 + 4, dw:dw + W]
                            rhsB = slabv[64:128, r0:r0 + 4, dw:dw + W]
                            po = half * Cout
                            # image A -> cols 0:448, image B -> 512:960
                            nc.tensor.matmul(
                                ps[g][po:po + Cout, 0:NCH],
                                wts[0:64, t, :], rhsA,
                                tile_position=(0, po), start=st, stop=sp)
                            nc.tensor.matmul(
                                ps[g][po:po + Cout, 512:512 + NCH],
                                wts[64:128, t, :], rhsB,
                                tile_position=(64, po), start=st, stop=sp)

                    def finish_round(g):
                        j = 2 * jj + g
                        z = zpool.tile([128, 960], f32, name="z")
                        nc.scalar.activation(
                            z[:], ps[g][:, 0:960],
                            mybir.ActivationFunctionType.Identity,
                            bias=bias05, scale=one)
                        nc.vector._custom_dve(
                            stair, out=oslab[:, j * PSW:j * PSW + 960],
                            in0=z[:], in1=hiconst, s0=rconst, s1=dconst,
                            imm2=float(MAGIC))

                    if s == NSLAB - 1 and jj == NRND // 2 - 1:
                        # tail: run the two rounds sequentially so round 12's
                        # activation/bucketize/DMA overlap round 13's matmuls
                        for g in range(2):
                            for t in range(NTAPS):
                                mm_tap(g, t, t == 0, t == NTAPS - 1)
                            finish_round(g)
                    else:
                        for t in range(NTAPS):
                            st, sp = (t == 0), (t == NTAPS - 1)
                            for g in range(2):     # round within pair
                                mm_tap(g, t, st, sp)
                        for g in range(2):
                            finish_round(g)

                # strided output DMA: image A = col block [0:448] of each
                # round, image B = [512:960]; junk cols never leave SBUF
                ov = oslab[:].rearrange("p (j c) -> p j c", c=PSW)
                jcuts = ([0, 4, 7, 10, 12, 13, NRND] if s == NSLAB - 1
                         else [0, 7, NRND])
                for lo, hi in zip(jcuts[:-1], jcuts[1:]):
                    nc.sync.dma_start(
                        out=out_t.ap()[2 * s, :, lo * NCH:hi * NCH],
                        in_=ov[:, lo:hi, 0:NCH])
                    nc.sync.dma_start(
                        out=out_t.ap()[2 * s + 1, :, lo * NCH:hi * NCH],
                        in_=ov[:, lo:hi, 512:512 + NCH])

    nc.compile()
    _built["nc"] = nc
    return nc


def _binarize_weights(w):
    """Exactly the reference's fp32 binarization. Returns (sign in {-1,0,1}, sw)."""
    w = np.asarray(w, np.float32)
    C = w.shape[0]
    wf = w.reshape(C, -1)
    bw = w - wf.mean(-1)[:, None, None, None]
    bw = bw / bw.reshape(C, -1).std(-1, ddof=1)[:, None, None, None]
    mean_abs = np.abs(bw).reshape(C, -1).mean(-1)
    sw = np.exp2(np.round(np.log2(mean_abs))).astype(np.float32)
    return np.sign(bw).astype(np.float32), sw


def _fit_lattice(t64):
    """Per channel find (u', d): lut_k in (u'+k*d-1, u'+k*d) ... i.e.
    lut_k < u' + k*d < lut_k + 1 for k=0..6, maximizing the margin.
    The LUT is round(u + k*d) of an arithmetic sequence, so this is feasible
    with margin ~0.25 generically. Returns (r, Dp, ok)."""
    k = np.arange(7)
    C = t64.shape[0]
    dd = np.arange(0.25, 4.0, 5e-5)                    # [M]
    lo = (t64[:, None, :] - k[None, None, :] * dd[None, :, None]).max(-1)
    hi = (t64[:, None, :] + 1 - k[None, None, :] * dd[None, :, None]).min(-1)
    marg = hi - lo                                      # [C, M]
    best = marg.argmax(1)
    m = marg[np.arange(C), best]
    d = dd[best]
    u = (lo[np.arange(C), best] + hi[np.arange(C), best]) / 2
    r = (1.0 / d).astype(np.float32)
    Dp = (0.5 - u / d).astype(np.float32)
    return r, Dp, m.min()


def kernel(x, w, lut):
    x = np.ascontiguousarray(np.asarray(x, np.float32))
    w = np.asarray(w, np.float32)
    lut = np.asarray(lut, np.float32)

    nc = _build()
    from concourse import bass_utils

    # ---- weights: binarize + fold the pow2 scale into the thresholds ----
    sgn, sw = _binarize_weights(w)                     # sgn [Cout,Cin,3,3]
    t64 = lut.astype(np.float64) / sw[:, None]         # [Cout,7] thresholds

    # The staircase formulation needs integer thresholds: t64 = lut/sw with
    # lut integer and sw a power of two. sw != 1 would make them non-integer;
    # verify (sw == 1 for the reference's BN stats; the conv scale then
    # cancels exactly).
    assert np.all(sw == 1.0), f"unexpected weight scale {np.unique(sw)}"
    assert np.all(t64 == np.round(t64)), "thresholds not integers"

    # lhsT per tap: wts[ci, t, co] = sgn[co, ci, dh, dw]; rows 64-127 (the
    # image-B row tiles) use the same weights
    wts = np.empty((128, NTAPS, Cout), np.float32)
    for t in range(NTAPS):
        wts[:Cin, t, :] = sgn[:, :, t // 3, t % 3].T
    wts[Cin:] = wts[:Cin]
    wts = wts.astype(np.float16)

    # ---- staircase params ----
    r, Dp, min_margin = _fit_lattice(t64)
    assert min_margin > 1e-3, f"lattice fit failed (margin {min_margin})"
    half = np.stack([r, Dp,
                     np.full(Cout, MAGIC + 7, np.float32),
                     np.full(Cout, 0.5, np.float32),
                     np.full(Cout, 1.0, np.float32)], axis=1)
    nrm = np.empty((128, 5), np.float32)
    nrm[:Cout] = half
    nrm[Cout:] = half

    # ---- single-fp16 images, zero-padded flat slabs, 2 images per slab ----
    hi = x.astype(np.float16)                          # [B, Cin, H, W]
    xin = np.zeros((B // 2, 128, SLABF), np.float16)
    view = xin.reshape(B // 2, 128, HP, WPAD)
    view[:, :Cin, 1:H + 1, 1:W + 1] = hi[0::2]
    view[:, Cin:, 1:H + 1, 1:W + 1] = hi[1::2]

    # ---- run on the 8 cores (SPMD, batch-sharded) ----
    wts_np = np.ascontiguousarray(wts)
    nrm_np = np.ascontiguousarray(nrm)
    in_maps = [
        {
            "xin": np.ascontiguousarray(xin[c * NSLAB:(c + 1) * NSLAB]),
            "wts": wts_np,
            "nrm": nrm_np,
        }
        for c in range(NCORES)
    ]
    try:
        res = bass_utils.run_bass_kernel_spmd(nc, in_maps,
                                              core_ids=list(range(NCORES)))
    except Exception:
        # transient PJRT/compile hiccups happen occasionally; retry once
        res = bass_utils.run_bass_kernel_spmd(nc, in_maps,
                                              core_ids=list(range(NCORES)))
    global last_results
    last_results = res

    # ---- unshard: interleave the adjacent 4-row chunks ----
    out = np.empty((B, Cout, H, W), np.float32)
    for c in range(NCORES):
        o = res.results[c]["out"]                      # [BPC, 128, HALF] u8
        top = o[:, :Cout, :].reshape(BPC, Cout, NRND, 4, W)
        bot = o[:, Cout:, :].reshape(BPC, Cout, NRND, 4, W)
        yrows = np.stack([top, bot], axis=3)           # [., ., 14, 2, 4, W]
        out[c * BPC:(c + 1) * BPC] = (
            yrows.reshape(BPC, Cout, H, W).astype(np.float32))
    return out
